# revision 2
# baseline (speedup 1.0000x reference)
"""DST-II kernel for Trainium2 (8 NeuronCores, Bass/Tile).

y[m, k] = sum_n x[m, n] * sin(pi/N * (n + 1/2) * (k + 1)),  x: [16384, 1024] f32.

This is a batched matmul y = x @ S with a fixed [1024, 1024] sine table.
Sharding: batch (rows of x) split across 8 cores, S replicated.

Device kernel (per core, M=2048 rows):
  - TensorE computes out = lhsT.T @ rhs where both operands carry the
    contraction dim on partitions, so the stationary operand must be x^T.
    We hand each core its row-slab already transposed (a host-side layout
    choice during sharding) -> clean natural-layout DMAs on device.
  - Matmuls run in float32r (TF32-like, full PE rate for free dim >= 256,
    ~1e-4 rel err). The BIR verifier requires fp32r operands to be produced
    by a rounding instruction, so inputs pass through a DVE rounding copy
    (overlapped with PE work).
"""

import numpy as np
from contextlib import ExitStack

import concourse.bass as bass
import concourse.mybir as mybir
import concourse.tile as tile
from concourse import bacc
from concourse.bass_utils import run_bass_kernel_spmd

N_CORES = 8
B = 16384            # total batch (rows)
N = 1024             # transform length
M_CORE = B // N_CORES   # rows per core = 2048
P = 128
KT = N // P          # 8 contraction tiles
M_CHUNK = 256        # rows processed per SBUF residency
N_HALF = 512         # matmul free dim (one PSUM bank)

_CACHE = {}


def _dst_table() -> np.ndarray:
    n = np.arange(N, dtype=np.float64)
    k = np.arange(N, dtype=np.float64)
    S = np.sin((np.pi / N) * (n[:, None] + 0.5) * (k[None, :] + 1.0))
    return S.astype(np.float32)


def _build():
    f32 = mybir.dt.float32
    f32r = mybir.dt.float32r
    nc = bacc.Bacc("TRN2", target_bir_lowering=False, debug=False,
                   enable_asserts=False)
    xT = nc.dram_tensor("xT", [N, M_CORE], f32, kind="ExternalInput").ap()
    S = nc.dram_tensor("S", [N, N], f32, kind="ExternalInput").ap()
    y = nc.dram_tensor("y", [M_CORE, N], f32, kind="ExternalOutput").ap()

    with tile.TileContext(nc) as tc:
        with ExitStack() as ctx:
            const = ctx.enter_context(tc.tile_pool(name="const", bufs=1))
            xin = ctx.enter_context(tc.tile_pool(name="xin", bufs=2))
            xrnd = ctx.enter_context(tc.tile_pool(name="xrnd", bufs=2))
            yout = ctx.enter_context(tc.tile_pool(name="yout", bufs=2))
            ps = ctx.enter_context(tc.tile_pool(name="ps", bufs=6, space="PSUM"))

            # S table: load once, round to f32r once.
            S_t = const.tile([P, KT, N], f32)
            nc.sync.dma_start(S_t[:], S.rearrange("(o p) f -> p o f", p=P))
            S_r = const.tile([P, KT, N], f32r)
            nc.vector.tensor_copy(out=S_r[:], in_=S_t[:])

            for c in range(M_CORE // M_CHUNK):
                m0 = c * M_CHUNK
                xc = xin.tile([P, KT, M_CHUNK], f32, tag="xc")
                nc.sync.dma_start(
                    xc[:],
                    xT[:, m0:m0 + M_CHUNK].rearrange("(o p) f -> p o f", p=P))
                xr = xrnd.tile([P, KT, M_CHUNK], f32r, tag="xr")
                nc.vector.tensor_copy(out=xr[:], in_=xc[:])

                yc = yout.tile([P, M_CHUNK // P, N], f32, tag="yc")
                for mt in range(M_CHUNK // P):
                    for h in range(N // N_HALF):
                        acc = ps.tile([P, N_HALF], f32, tag="acc")
                        for k in range(KT):
                            nc.tensor.matmul(
                                acc[:],
                                xr[:, k, mt * P:(mt + 1) * P],
                                S_r[:, k, h * N_HALF:(h + 1) * N_HALF],
                                start=(k == 0), stop=(k == KT - 1))
                        nc.vector.tensor_copy(
                            out=yc[:, mt, h * N_HALF:(h + 1) * N_HALF],
                            in_=acc[:])
                nc.sync.dma_start(
                    y[m0:m0 + M_CHUNK, :].rearrange("(o p) f -> p o f", p=P),
                    yc[:])

    nc.compile()
    return nc


def _get_nc():
    if "nc" not in _CACHE:
        _CACHE["nc"] = _build()
    return _CACHE["nc"]


def _in_maps(x: np.ndarray):
    S = _CACHE.setdefault("S", _dst_table())
    x = np.ascontiguousarray(x, dtype=np.float32)
    maps = []
    for c in range(N_CORES):
        xs = x[c * M_CORE:(c + 1) * M_CORE]
        maps.append({"xT": np.ascontiguousarray(xs.T), "S": S})
    return maps


def kernel(x: np.ndarray) -> np.ndarray:
    nc = _get_nc()
    res = run_bass_kernel_spmd(nc, _in_maps(x), list(range(N_CORES)))
    return np.concatenate([res.results[c]["y"] for c in range(N_CORES)], axis=0)


def _install_profile_hooks():
    """The agent image's antenv lacks axon_hooks; recreate it from
    trn_agent_boot so run_bass_kernel_spmd(trace=True) can capture NTFF
    profiles. Also stub out the S3 artifact upload."""
    import sys, types
    import concourse.bass_utils as bu

    if "antenv.axon_hooks" not in sys.modules:
        from trn_agent_boot.trn_boot import _ntff_profile_via_ctypes
        hook = _ntff_profile_via_ctypes("/opt/axon/libaxon_pjrt.so")
        mod = types.ModuleType("antenv.axon_hooks")
        mod.get_axon_ntff_profile_hook = lambda: hook
        mod.set_axon_ntff_profile_hook = lambda h: None
        sys.modules["antenv.axon_hooks"] = mod
    bu.upload_artifacts = lambda tmpdir: f"local:{tmpdir}"


def profile(x: np.ndarray, tmpdir=None, trace_kwargs={}):
    """Run once with NTFF tracing; returns (exec_time_ns, BassKernelResults)."""
    _install_profile_hooks()
    nc = _get_nc()
    res = run_bass_kernel_spmd(nc, _in_maps(x), list(range(N_CORES)),
                               trace=True, tmpdir=tmpdir,
                               trace_kwargs=trace_kwargs)
    return res.exec_time_ns, res


# revision 3
# speedup vs baseline: 1.0194x; 1.0194x over previous
"""DST-II kernel for Trainium2 (8 NeuronCores, Bass/Tile).

y[m, k] = sum_n x[m, n] * sin(pi/N * (n + 1/2) * (k + 1)),  x: [16384, 1024] f32.

This is a batched matmul y = x @ S with a fixed [1024, 1024] sine table.
Sharding: batch (rows of x) split across 8 cores, S replicated.

Device kernel (per core, M=2048 rows):
  - TensorE computes out = lhsT.T @ rhs where both operands carry the
    contraction dim on partitions, so the stationary operand must be x^T.
    We hand each core its row-slab already transposed (a host-side layout
    choice during sharding) -> clean natural-layout DMAs on device.
  - Matmuls run in float32r (TF32-like, full PE rate for free dim >= 256,
    ~1.4e-4 rel err). Inputs are declared float32r in DRAM directly —
    hardware accepts raw fp32 bits with identical accuracy to pre-rounded
    data, so no DVE rounding pass is needed.
"""

import numpy as np
from contextlib import ExitStack

import concourse.bass as bass
import concourse.mybir as mybir
import concourse.tile as tile
from concourse import bacc
from concourse.bass_utils import run_bass_kernel_spmd

N_CORES = 8
B = 16384            # total batch (rows)
N = 1024             # transform length
M_CORE = B // N_CORES   # rows per core = 2048
P = 128
KT = N // P          # 8 contraction tiles
M_CHUNK = 256        # rows processed per SBUF residency
N_HALF = 512         # matmul free dim (one PSUM bank)

_CACHE = {}


def _dst_table() -> np.ndarray:
    n = np.arange(N, dtype=np.float64)
    k = np.arange(N, dtype=np.float64)
    S = np.sin((np.pi / N) * (n[:, None] + 0.5) * (k[None, :] + 1.0))
    return S.astype(np.float32)


def _build():
    f32 = mybir.dt.float32
    f32r = mybir.dt.float32r
    nc = bacc.Bacc("TRN2", target_bir_lowering=False, debug=False,
                   enable_asserts=False)
    xT = nc.dram_tensor("xT", [N, M_CORE], f32r, kind="ExternalInput").ap()
    S = nc.dram_tensor("S", [N, N], f32r, kind="ExternalInput").ap()
    y = nc.dram_tensor("y", [M_CORE, N], f32, kind="ExternalOutput").ap()

    n_chunks = M_CORE // M_CHUNK

    with tile.TileContext(nc) as tc:
        with ExitStack() as ctx:
            const = ctx.enter_context(tc.tile_pool(name="const", bufs=1))
            xin = ctx.enter_context(tc.tile_pool(name="xin", bufs=4))
            yout = ctx.enter_context(tc.tile_pool(name="yout", bufs=3))
            ps = ctx.enter_context(tc.tile_pool(name="ps", bufs=6, space="PSUM"))

            # S table in two kout-halves so the first matmuls only wait on
            # a 2MB transfer.
            S_half = []
            for h in range(N // N_HALF):
                t = const.tile([P, KT, N_HALF], f32r, tag=f"S{h}")
                nc.sync.dma_start(
                    t[:],
                    S[:, h * N_HALF:(h + 1) * N_HALF]
                    .rearrange("(o p) f -> p o f", p=P))
                S_half.append(t)

            for c in range(n_chunks):
                m0 = c * M_CHUNK
                xc = xin.tile([P, KT, M_CHUNK], f32r, tag="xc")
                nc.sync.dma_start(
                    xc[:],
                    xT[:, m0:m0 + M_CHUNK].rearrange("(o p) f -> p o f", p=P))

                yc = yout.tile([P, M_CHUNK // P, N], f32, tag="yc")
                for mt in range(M_CHUNK // P):
                    for h in range(N // N_HALF):
                        acc = ps.tile([P, N_HALF], f32, tag="acc")
                        for k in range(KT):
                            nc.tensor.matmul(
                                acc[:],
                                xc[:, k, mt * P:(mt + 1) * P],
                                S_half[h][:, k, :],
                                start=(k == 0), stop=(k == KT - 1))
                        nc.vector.tensor_copy(
                            out=yc[:, mt, h * N_HALF:(h + 1) * N_HALF],
                            in_=acc[:])
                nc.sync.dma_start(
                    y[m0:m0 + M_CHUNK, :].rearrange("(o p) f -> p o f", p=P),
                    yc[:])

    nc.compile()
    return nc


def _get_nc():
    if "nc" not in _CACHE:
        _CACHE["nc"] = _build()
    return _CACHE["nc"]


def _in_maps(x: np.ndarray):
    S = _CACHE.setdefault("S", _dst_table())
    x = np.ascontiguousarray(x, dtype=np.float32)
    maps = []
    for c in range(N_CORES):
        xs = x[c * M_CORE:(c + 1) * M_CORE]
        maps.append({"xT": np.ascontiguousarray(xs.T), "S": S})
    return maps


def kernel(x: np.ndarray) -> np.ndarray:
    nc = _get_nc()
    res = run_bass_kernel_spmd(nc, _in_maps(x), list(range(N_CORES)))
    return np.concatenate([res.results[c]["y"] for c in range(N_CORES)], axis=0)


def _install_profile_hooks():
    """The agent image's antenv lacks axon_hooks; recreate it from
    trn_agent_boot so run_bass_kernel_spmd(trace=True) can capture NTFF
    profiles. Also stub out the S3 artifact upload."""
    import sys, types
    import concourse.bass_utils as bu

    if "antenv.axon_hooks" not in sys.modules:
        from trn_agent_boot.trn_boot import _ntff_profile_via_ctypes
        hook = _ntff_profile_via_ctypes("/opt/axon/libaxon_pjrt.so")
        mod = types.ModuleType("antenv.axon_hooks")
        mod.get_axon_ntff_profile_hook = lambda: hook
        mod.set_axon_ntff_profile_hook = lambda h: None
        sys.modules["antenv.axon_hooks"] = mod
    bu.upload_artifacts = lambda tmpdir: f"local:{tmpdir}"


def profile(x: np.ndarray, tmpdir=None, trace_kwargs={}):
    """Run once with NTFF tracing; returns (exec_time_ns, BassKernelResults)."""
    _install_profile_hooks()
    nc = _get_nc()
    res = run_bass_kernel_spmd(nc, _in_maps(x), list(range(N_CORES)),
                               trace=True, tmpdir=tmpdir,
                               trace_kwargs=trace_kwargs)
    return res.exec_time_ns, res


# revision 4
# speedup vs baseline: 1.2185x; 1.1954x over previous
"""DST-II kernel for Trainium2 (8 NeuronCores, Bass/Tile).

y[m, k] = sum_n x[m, n] * sin(pi/N * (n + 1/2) * (k + 1)),  x: [16384, 1024] f32.

This is a batched matmul y = x @ S with a fixed [1024, 1024] sine table.
Sharding: batch (rows of x) split across 8 cores, S replicated.

Fold-1 optimization: the sine table has the row symmetry
S[N-1-n, k] = (-1)^k S[n, k], so with u = x[:, :512] + x[:, :511:-1] and
v = x[:, :512] - x[:, :511:-1]:
    y[:, 0::2] = u @ A,  A = S[:512, 0::2]   (512x512)
    y[:, 1::2] = v @ B,  B = S[:512, 1::2]   (512x512)
which halves both the matmul FLOPs and the table traffic. The fold adds run
on the vector engine in fp32.

Device kernel (per core, M=2048 rows):
  - TensorE computes out = lhsT.T @ rhs where both operands carry the
    contraction dim on partitions, so the stationary operand must be u^T/v^T.
    We hand each core its row-slab pre-transposed with the second half of
    the columns reversed (host-side layout choice during sharding), so the
    fold is a plain tile add/sub on device.
  - Matmuls run in float32r (TF32-like, 2 cycles/row for 4-byte operands,
    ~1.4e-4 rel err). Inputs are declared float32r in DRAM directly — the
    hardware accepts raw fp32 bits with accuracy identical to pre-rounded
    data, so no rounding pass is needed.
  - Chunk sizes ramp 128..512..128 so the first matmul isn't gated on a
    large DMA and the final store is small.
"""

import numpy as np
from contextlib import ExitStack

import concourse.bass as bass
import concourse.mybir as mybir
import concourse.tile as tile
from concourse import bacc
from concourse.bass_utils import run_bass_kernel_spmd

N_CORES = 8
B = 16384            # total batch (rows)
N = 1024             # transform length
M_CORE = B // N_CORES   # rows per core = 2048
P = 128
NH = N // 2          # folded contraction length = 512
KT = NH // P         # 4 contraction tiles per branch
N_FREE = 512         # matmul free dim (one PSUM bank)
CHUNKS = [128, 128, 256, 512, 512, 256, 128, 128]
MAX_CHUNK = max(CHUNKS)
assert sum(CHUNKS) == M_CORE

_CACHE = {}


def _dst_table() -> np.ndarray:
    n = np.arange(N, dtype=np.float64)
    k = np.arange(N, dtype=np.float64)
    S = np.sin((np.pi / N) * (n[:, None] + 0.5) * (k[None, :] + 1.0))
    return S.astype(np.float32)


def _tables() -> np.ndarray:
    """[2, 512, 512]: A = S[:512, 0::2], B = S[:512, 1::2]."""
    S = _dst_table()
    return np.ascontiguousarray(
        np.stack([S[:NH, 0::2], S[:NH, 1::2]], axis=0))


def _build():
    f32 = mybir.dt.float32
    f32r = mybir.dt.float32r
    nc = bacc.Bacc("TRN2", target_bir_lowering=False, debug=False,
                   enable_asserts=False)
    # xT2 rows: 0..511 = x cols 0..511; 512..1023 = x cols 1023..512 (reversed)
    xT2 = nc.dram_tensor("xT2", [N, M_CORE], f32r, kind="ExternalInput").ap()
    AB = nc.dram_tensor("AB", [2, NH, NH], f32r, kind="ExternalInput").ap()
    y = nc.dram_tensor("y", [M_CORE, N], f32, kind="ExternalOutput").ap()

    with tile.TileContext(nc) as tc:
        with ExitStack() as ctx:
            const = ctx.enter_context(tc.tile_pool(name="const", bufs=1))
            xin = ctx.enter_context(tc.tile_pool(name="xin", bufs=3))
            fold = ctx.enter_context(tc.tile_pool(name="fold", bufs=2))
            yout = ctx.enter_context(tc.tile_pool(name="yout", bufs=2))
            ps = ctx.enter_context(tc.tile_pool(name="ps", bufs=6, space="PSUM"))

            # Tables as 8 small [128, 512] tiles (k-tile granularity) so the
            # first matmuls only gate on a 256KB transfer.
            tabs = {}
            for k in range(KT):
                for t in range(2):
                    tt = const.tile([P, N_FREE], f32r, tag=f"tab{t}_{k}")
                    nc.sync.dma_start(
                        tt[:], AB[t, k * P:(k + 1) * P, :])
                    tabs[(t, k)] = tt

            m0 = 0
            for mc in CHUNKS:
                xc = xin.tile([P, 2 * KT, MAX_CHUNK], f32r, tag="xc")
                nc.sync.dma_start(
                    xc[:, :, :mc],
                    xT2[:, m0:m0 + mc].rearrange("(o p) f -> p o f", p=P))
                u = fold.tile([P, KT, MAX_CHUNK], f32r, tag="u")
                v = fold.tile([P, KT, MAX_CHUNK], f32r, tag="v")
                nc.vector.tensor_add(u[:, :, :mc], xc[:, :KT, :mc],
                                     xc[:, KT:, :mc])
                nc.vector.tensor_sub(v[:, :, :mc], xc[:, :KT, :mc],
                                     xc[:, KT:, :mc])

                yc = yout.tile([P, MAX_CHUNK // P, N], f32, tag="yc")
                for mt in range(mc // P):
                    ym = yc[:, mt, :].rearrange("p (i two) -> p i two", two=2)
                    for t, src in ((0, u), (1, v)):
                        acc = ps.tile([P, N_FREE], f32, tag="acc")
                        for k in range(KT):
                            nc.tensor.matmul(
                                acc[:],
                                src[:, k, mt * P:(mt + 1) * P],
                                tabs[(t, k)][:],
                                start=(k == 0), stop=(k == KT - 1))
                        nc.vector.tensor_copy(out=ym[:, :, t], in_=acc[:])
                nc.sync.dma_start(
                    y[m0:m0 + mc, :].rearrange("(o p) f -> p o f", p=P),
                    yc[:, :mc // P, :])
                m0 += mc

    nc.compile()
    return nc


def _get_nc():
    if "nc" not in _CACHE:
        _CACHE["nc"] = _build()
    return _CACHE["nc"]


def _in_maps(x: np.ndarray):
    AB = _CACHE.setdefault("AB", _tables())
    x = np.ascontiguousarray(x, dtype=np.float32)
    maps = []
    for c in range(N_CORES):
        xs = x[c * M_CORE:(c + 1) * M_CORE]
        xT2 = np.ascontiguousarray(
            np.concatenate([xs[:, :NH], xs[:, :NH - 1:-1]], axis=1).T)
        maps.append({"xT2": xT2, "AB": AB})
    return maps


def kernel(x: np.ndarray) -> np.ndarray:
    nc = _get_nc()
    res = run_bass_kernel_spmd(nc, _in_maps(x), list(range(N_CORES)))
    return np.concatenate([res.results[c]["y"] for c in range(N_CORES)], axis=0)


def _install_profile_hooks():
    """The agent image's antenv lacks axon_hooks; recreate it from
    trn_agent_boot so run_bass_kernel_spmd(trace=True) can capture NTFF
    profiles. Also stub out the S3 artifact upload."""
    import sys, types
    import concourse.bass_utils as bu

    if "antenv.axon_hooks" not in sys.modules:
        from trn_agent_boot.trn_boot import _ntff_profile_via_ctypes
        hook = _ntff_profile_via_ctypes("/opt/axon/libaxon_pjrt.so")
        mod = types.ModuleType("antenv.axon_hooks")
        mod.get_axon_ntff_profile_hook = lambda: hook
        mod.set_axon_ntff_profile_hook = lambda h: None
        sys.modules["antenv.axon_hooks"] = mod
    bu.upload_artifacts = lambda tmpdir: f"local:{tmpdir}"


def profile(x: np.ndarray, tmpdir=None, trace_kwargs={}):
    """Run once with NTFF tracing; returns (exec_time_ns, BassKernelResults)."""
    _install_profile_hooks()
    nc = _get_nc()
    res = run_bass_kernel_spmd(nc, _in_maps(x), list(range(N_CORES)),
                               trace=True, tmpdir=tmpdir,
                               trace_kwargs=trace_kwargs)
    return res.exec_time_ns, res


# revision 7
# speedup vs baseline: 1.3183x; 1.0819x over previous
"""DST-II kernel for Trainium2 (8 NeuronCores, Bass/Tile).

y[m, k] = sum_n x[m, n] * sin(pi/N * (n + 1/2) * (k + 1)),  x: [16384, 1024] f32.

This is a batched matmul y = x @ S with a fixed [1024, 1024] sine table.
Sharding: batch (rows of x) split across 8 cores, S replicated.

Fold-1 optimization: the sine table has the row symmetry
S[N-1-n, k] = (-1)^k S[n, k], so with u = x[:, :512] + x[:, :511:-1] and
v = x[:, :512] - x[:, :511:-1]:
    y[:, 0::2] = u @ A,  A = S[:512, 0::2]   (512x512)
    y[:, 1::2] = v @ B,  B = S[:512, 1::2]   (512x512)
which halves both the matmul FLOPs and the table traffic. The fold adds run
on the vector engine in fp32.

Device kernel (per core, M=2048 rows):
  - TensorE computes out = lhsT.T @ rhs where both operands carry the
    contraction dim on partitions, so the stationary operand must be u^T/v^T.
    We hand each core its row-slab pre-transposed with the second half of
    the columns reversed (host-side layout choice during sharding), so the
    fold is a plain tile add/sub on device.
  - Matmuls run in float32r (TF32-like, 2 cycles/row for 4-byte operands,
    ~1.4e-4 rel err). Inputs are declared float32r in DRAM directly — the
    hardware accepts raw fp32 bits with accuracy identical to pre-rounded
    data, so no rounding pass is needed.
  - Chunk sizes ramp 128..512..128 so the first matmul isn't gated on a
    large DMA and the final store is small.
"""

import numpy as np
from contextlib import ExitStack

import concourse.bass as bass
import concourse.mybir as mybir
import concourse.tile as tile
from concourse import bacc
from concourse.bass_utils import run_bass_kernel_spmd

N_CORES = 8
B = 16384            # total batch (rows)
N = 1024             # transform length
M_CORE = B // N_CORES   # rows per core = 2048
P = 128
NH = N // 2          # folded contraction length = 512
KT = NH // P         # 4 contraction tiles per branch
N_FREE = 512         # matmul free dim (one PSUM bank)
CHUNKS = [128, 128, 256, 512, 512, 256, 128, 128]
MAX_CHUNK = max(CHUNKS)
assert sum(CHUNKS) == M_CORE

_CACHE = {}


def _dst_table() -> np.ndarray:
    n = np.arange(N, dtype=np.float64)
    k = np.arange(N, dtype=np.float64)
    S = np.sin((np.pi / N) * (n[:, None] + 0.5) * (k[None, :] + 1.0))
    return S.astype(np.float32)


def _tables() -> np.ndarray:
    """[2, 512, 512]: A = S[:512, 0::2], B = S[:512, 1::2]."""
    S = _dst_table()
    return np.ascontiguousarray(
        np.stack([S[:NH, 0::2], S[:NH, 1::2]], axis=0))


def _build():
    f32 = mybir.dt.float32
    f32r = mybir.dt.float32r
    nc = bacc.Bacc("TRN2", target_bir_lowering=False, debug=False,
                   enable_asserts=False)
    # xT2 rows: 0..511 = x cols 0..511; 512..1023 = x cols 1023..512 (reversed)
    xT2 = nc.dram_tensor("xT2", [N, M_CORE], f32r, kind="ExternalInput").ap()
    AB = nc.dram_tensor("AB", [2, NH, NH], f32r, kind="ExternalInput").ap()
    y = nc.dram_tensor("y", [M_CORE, N], f32, kind="ExternalOutput").ap()

    with tile.TileContext(nc) as tc:
        with ExitStack() as ctx:
            const = ctx.enter_context(tc.tile_pool(name="const", bufs=1))
            xin = ctx.enter_context(tc.tile_pool(name="xin", bufs=4))
            fold = ctx.enter_context(tc.tile_pool(name="fold", bufs=2))
            yout = ctx.enter_context(tc.tile_pool(name="yout", bufs=2))
            ps = ctx.enter_context(tc.tile_pool(name="ps", bufs=6, space="PSUM"))

            # Tables as 8 small [128, 512] tiles (k-tile granularity). The A
            # tiles (needed by the first psum group) are issued before the
            # first x chunk; the B tiles after it, hidden under the u-branch
            # matmuls.
            tabs = {}

            def load_tab(t, k):
                tt = const.tile([P, N_FREE], f32r, tag=f"tab{t}_{k}")
                nc.sync.dma_start(tt[:], AB[t, k * P:(k + 1) * P, :])
                tabs[(t, k)] = tt

            for k in range(KT):
                load_tab(0, k)

            m0 = 0
            for ci, mc in enumerate(CHUNKS):
                xc = xin.tile([P, 2 * KT, MAX_CHUNK], f32r, tag="xc")
                nc.sync.dma_start(
                    xc[:, :, :mc],
                    xT2[:, m0:m0 + mc].rearrange("(o p) f -> p o f", p=P))
                if ci == 0:
                    for k in range(KT):
                        load_tab(1, k)
                u = fold.tile([P, KT, MAX_CHUNK], f32r, tag="u")
                v = fold.tile([P, KT, MAX_CHUNK], f32r, tag="v")
                nc.vector.tensor_add(u[:, :, :mc], xc[:, :KT, :mc],
                                     xc[:, KT:, :mc])
                nc.vector.tensor_sub(v[:, :, :mc], xc[:, :KT, :mc],
                                     xc[:, KT:, :mc])

                yc = yout.tile([P, MAX_CHUNK // P, N], f32, tag="yc")
                for mt in range(mc // P):
                    ym = yc[:, mt, :].rearrange("p (i two) -> p i two", two=2)
                    for t, src in ((0, u), (1, v)):
                        acc = ps.tile([P, N_FREE], f32, tag="acc")
                        for k in range(KT):
                            nc.tensor.matmul(
                                acc[:],
                                src[:, k, mt * P:(mt + 1) * P],
                                tabs[(t, k)][:],
                                start=(k == 0), stop=(k == KT - 1))
                        nc.scalar.copy(out=ym[:, :, t], in_=acc[:])
                nc.sync.dma_start(
                    y[m0:m0 + mc, :].rearrange("(o p) f -> p o f", p=P),
                    yc[:, :mc // P, :])
                m0 += mc

    nc.compile()
    return nc


def _get_nc():
    if "nc" not in _CACHE:
        _CACHE["nc"] = _build()
    return _CACHE["nc"]


def _in_maps(x: np.ndarray):
    AB = _CACHE.setdefault("AB", _tables())
    x = np.ascontiguousarray(x, dtype=np.float32)
    maps = []
    for c in range(N_CORES):
        xs = x[c * M_CORE:(c + 1) * M_CORE]
        xT2 = np.ascontiguousarray(
            np.concatenate([xs[:, :NH], xs[:, :NH - 1:-1]], axis=1).T)
        maps.append({"xT2": xT2, "AB": AB})
    return maps


def kernel(x: np.ndarray) -> np.ndarray:
    nc = _get_nc()
    res = run_bass_kernel_spmd(nc, _in_maps(x), list(range(N_CORES)))
    return np.concatenate([res.results[c]["y"] for c in range(N_CORES)], axis=0)


def _install_profile_hooks():
    """The agent image's antenv lacks axon_hooks; recreate it from
    trn_agent_boot so run_bass_kernel_spmd(trace=True) can capture NTFF
    profiles. Also stub out the S3 artifact upload."""
    import sys, types
    import concourse.bass_utils as bu

    if "antenv.axon_hooks" not in sys.modules:
        from trn_agent_boot.trn_boot import _ntff_profile_via_ctypes
        hook = _ntff_profile_via_ctypes("/opt/axon/libaxon_pjrt.so")
        mod = types.ModuleType("antenv.axon_hooks")
        mod.get_axon_ntff_profile_hook = lambda: hook
        mod.set_axon_ntff_profile_hook = lambda h: None
        sys.modules["antenv.axon_hooks"] = mod
    bu.upload_artifacts = lambda tmpdir: f"local:{tmpdir}"


def profile(x: np.ndarray, tmpdir=None, trace_kwargs={}):
    """Run once with NTFF tracing; returns (exec_time_ns, BassKernelResults)."""
    _install_profile_hooks()
    nc = _get_nc()
    res = run_bass_kernel_spmd(nc, _in_maps(x), list(range(N_CORES)),
                               trace=True, tmpdir=tmpdir,
                               trace_kwargs=trace_kwargs)
    return res.exec_time_ns, res
